# revision 9
# baseline (speedup 1.0000x reference)
"""MultiLabelContrastiveFocalLoss on 8 Trainium2 NeuronCores.

Math
----
loss = mean(focal) + contrastive, where (t in {0,1}, p = sigmoid(x), s = 1-p)
  focal_elem   = ALPHA * s^2 * (softplus(x) - x*t),  softplus(x) = -log(s)
  contrastive  = sum_{i!=j} (1 - <t_i,t_j>) <p_i,p_j> / (B*(B-1))
               = (||u||^2 - sum(p^2) - ||T^T P||_F^2 + sum_i ||t_i||^2 ||p_i||^2) / D
  with u = column-sums of P, D = B*(B-1).

Sharding (8 cores, SPMD, uniform program)
-----------------------------------------
The L=2048 columns split into eight 256-col blocks. Core c (r = c//4, q = c%4):
  - x-cols   = quarter q  (blocks 2q, 2q+1), matching block 2q+r placed first
  - t-cols   = Tset_r     (blocks with parity r), matching block 2q+r first
  - computes the [1024, 512] block of M = T^T P (rows Tset_r, cols quarter q)
  - focal on x-block 2q+r vs t-block 2q+r (each block covered exactly once)
Each core outputs raw partial scalars [f, p2, d, m2, u2]; the host gather
(unshard) step combines them with fixed weights into the final scalar.
"""

import numpy as np

import concourse.bacc as bacc
import concourse.bass as bass  # noqa: F401
import concourse.mybir as mybir
import concourse.tile as tile
from concourse.bass_utils import run_bass_kernel_spmd

mm = mybir.dt
AF = mybir.ActivationFunctionType
ALU = mybir.AluOpType

B, L = 4096, 2048
ALPHA = 0.25
N_CORES = 8
KT = B // 128          # 32 k-tiles over rows
XC = L // 4            # 512  x-cols per core
TC = L // 2            # 1024 t-cols per core
FC = 256               # focal cols per core
MT = TC // 128         # 8 m-tiles
HALVES = 2             # ln-batch chunking of the k loop
S_EPS = 1.001          # s = S_EPS - p  (guards log(0) at bf16 precision)

_CACHE: dict = {}


def build_nc(*, with_focal=True, with_u=True, mm_order="k_outer", loop_n=None):
    nc = bacc.Bacc("TRN2", target_bir_lowering=False, debug=False,
                   num_devices=N_CORES)
    xq_ext = nc.dram_tensor("xq", [B, XC], mm.float32, kind="ExternalInput")
    th_ext = nc.dram_tensor("th", [B, TC], mm.float32, kind="ExternalInput")
    out_ext = nc.dram_tensor("out", [1, 8], mm.float32, kind="ExternalOutput")

    xq_t = xq_ext.ap().rearrange("(k p) n -> k p n", p=128)
    th_t = th_ext.ap().rearrange("(k p) n -> k p n", p=128)

    with tile.TileContext(nc) as tc:
        with (
            tc.tile_pool(name="xstage", bufs=3) as xstage_pool,
            tc.tile_pool(name="tstage", bufs=3) as tstage_pool,
            tc.tile_pool(name="tb", bufs=KT) as tb_pool,
            tc.tile_pool(name="pb", bufs=KT) as pb_pool,
            tc.tile_pool(name="sb", bufs=KT) as sb_pool,
            tc.tile_pool(name="scr", bufs=2) as scr_pool,
            tc.tile_pool(name="fb", bufs=3) as fb_pool,
            tc.tile_pool(name="stats", bufs=1) as stats_pool,
            tc.tile_pool(name="ps", bufs=8, space="PSUM") as ps_pool,
        ):
            def emit_body():
                rowT2 = stats_pool.tile([128, KT], mm.float32, tag="rowT2")
                rowP2 = stats_pool.tile([128, KT], mm.float32, tag="rowP2")
                fst = stats_pool.tile([128, KT], mm.float32, tag="fst")
                if not with_focal:
                    nc.vector.memset(fst[:], 0.0)
                m2st = stats_pool.tile([128, MT], mm.float32, tag="m2st")
                if mm_order == "none":
                    nc.vector.memset(m2st[:], 0.0)
                stats2 = stats_pool.tile([128, 4], mm.float32, tag="stats2")
                ones_bf = stats_pool.tile([128, 1], mm.bfloat16, tag="ones_bf")
                ones_f32 = stats_pool.tile([128, 1], mm.float32, tag="ones_f32")
                nc.vector.memset(ones_bf[:], 1.0)
                nc.vector.memset(ones_f32[:], 1.0)

                psA = [ps_pool.tile([128, XC], mm.float32, tag="bank",
                                    name=f"psA{m}") for m in range(MT)]

                tb = [None] * KT
                pb = [None] * KT
                sb = [None] * KT
                s2b = [None] * KT
                xtb = [None] * KT

                ksplit = [range(h * KT // HALVES, (h + 1) * KT // HALVES)
                          for h in range(HALVES)]

                for half in range(HALVES):
                    # ---- phase A: load, sigmoid, casts, matmul ----
                    for k in ksplit[half]:
                        tstage = tstage_pool.tile([128, TC], mm.float32)
                        nc.sync.dma_start(out=tstage[:], in_=th_t[k])
                        tb[k] = tb_pool.tile([128, TC], mm.bfloat16,
                                             name=f"tb{k}", tag="tb")
                        nc.vector.tensor_scalar(
                            out=tb[k][:], in0=tstage[:], scalar1=1.0, scalar2=0.0,
                            op0=ALU.mult, op1=ALU.add,
                            accum_out=rowT2[:, k:k + 1])

                        xstage = xstage_pool.tile([128, XC], mm.float32)
                        nc.sync.dma_start(out=xstage[:], in_=xq_t[k])
                        pb[k] = pb_pool.tile([128, XC], mm.bfloat16,
                                             name=f"pb{k}", tag="pb")
                        nc.scalar.activation(pb[k][:], xstage[:], AF.Sigmoid)
                        scr = scr_pool.tile([128, XC], mm.float32, tag="sq")
                        nc.scalar.activation(scr[:], pb[k][:], AF.Square,
                                             accum_out=rowP2[:, k:k + 1])
                        if with_focal:
                            # s = S_EPS - p, s^2, x*t  (all overlapped w/ DMA)
                            sb[k] = sb_pool.tile([128, FC], mm.bfloat16,
                                                 name=f"sb{k}", tag="sb")
                            nc.vector.tensor_scalar(
                                out=sb[k][:], in0=pb[k][:, 0:FC], scalar1=-1.0,
                                scalar2=S_EPS, op0=ALU.mult, op1=ALU.add)
                            s2b[k] = sb_pool.tile([128, FC], mm.bfloat16,
                                                  name=f"s2b{k}", tag="s2b")
                            nc.vector.tensor_tensor(
                                out=s2b[k][:], in0=sb[k][:], in1=sb[k][:],
                                op=ALU.mult)
                            xtb[k] = sb_pool.tile([128, FC], mm.bfloat16,
                                                  name=f"xtb{k}", tag="xtb")
                            nc.vector.tensor_tensor(
                                out=xtb[k][:], in0=xstage[:, 0:FC],
                                in1=tb[k][:, 0:FC], op=ALU.mult)

                        if mm_order == "k_outer":
                            for m in range(MT):
                                nc.tensor.matmul(
                                    psA[m][:],
                                    tb[k][:, 128 * m:128 * (m + 1)], pb[k][:],
                                    start=(k == 0), stop=(k == KT - 1))

                    if half == HALVES - 1:
                        # ---- drains + u-sweep before the last ln batch ----
                        if mm_order == "m_outer":
                            for m in range(MT):
                                for k in range(KT):
                                    nc.tensor.matmul(
                                        psA[m][:],
                                        tb[k][:, 128 * m:128 * (m + 1)],
                                        pb[k][:],
                                        start=(k == 0), stop=(k == KT - 1))
                        if mm_order != "none":
                            for m in range(MT):
                                scr = scr_pool.tile([128, XC], mm.float32,
                                                    tag="sq")
                                nc.scalar.activation(
                                    scr[:], psA[m][:], AF.Square,
                                    accum_out=m2st[:, m:m + 1])
                        u2sb = stats_pool.tile([1, 1], mm.float32, tag="u2")
                        if with_u:
                            psU = ps_pool.tile([1, XC], mm.float32, tag="bank")
                            for k in range(KT):
                                nc.tensor.matmul(psU[:], ones_bf[:], pb[k][:],
                                                 start=(k == 0),
                                                 stop=(k == KT - 1))
                            uscr = scr_pool.tile([1, XC], mm.float32, tag="usq")
                            nc.scalar.activation(uscr[:], psU[:], AF.Square,
                                                 accum_out=u2sb[:])
                        else:
                            nc.vector.memset(u2sb[:], 0.0)

                    # ---- phase B: focal (ACT switches to natural_log set) ----
                    for k in (ksplit[half] if with_focal else []):
                        lns = fb_pool.tile([128, FC], mm.bfloat16, tag="lns")
                        nc.scalar.activation(lns[:], sb[k][:], AF.Ln)
                        bce = fb_pool.tile([128, FC], mm.bfloat16, tag="bce")
                        nc.vector.scalar_tensor_tensor(
                            out=bce[:], in0=lns[:], scalar=-1.0, in1=xtb[k][:],
                            op0=ALU.mult, op1=ALU.subtract)
                        fscr = fb_pool.tile([128, FC], mm.float32, tag="fscr")
                        nc.vector.scalar_tensor_tensor(
                            out=fscr[:], in0=s2b[k][:], scalar=1.0, in1=bce[:],
                            op0=ALU.mult, op1=ALU.mult,
                            accum_out=fst[:, k:k + 1])

                # ---- reduce stats to [128,4], then partition 0 via matmul ----
                scr32 = scr_pool.tile([128, KT], mm.float32, tag="r32")
                nc.vector.tensor_scalar(
                    out=scr32[:], in0=fst[:], scalar1=1.0, scalar2=0.0,
                    op0=ALU.mult, op1=ALU.add, accum_out=stats2[:, 0:1])
                scr32b = scr_pool.tile([128, KT], mm.float32, tag="r32")
                nc.vector.tensor_scalar(
                    out=scr32b[:], in0=rowP2[:], scalar1=1.0, scalar2=0.0,
                    op0=ALU.mult, op1=ALU.add, accum_out=stats2[:, 1:2])
                scr32c = scr_pool.tile([128, KT], mm.float32, tag="r32")
                nc.vector.scalar_tensor_tensor(
                    out=scr32c[:], in0=rowT2[:], scalar=1.0, in1=rowP2[:],
                    op0=ALU.mult, op1=ALU.mult, accum_out=stats2[:, 2:3])
                scr8 = scr_pool.tile([128, MT], mm.float32, tag="r8")
                nc.vector.tensor_scalar(
                    out=scr8[:], in0=m2st[:], scalar1=1.0, scalar2=0.0,
                    op0=ALU.mult, op1=ALU.add, accum_out=stats2[:, 3:4])

                psF = ps_pool.tile([1, 4], mm.float32, tag="bank")
                nc.tensor.matmul(psF[:], ones_f32[:], stats2[:],
                                 start=True, stop=True)

                osb = stats_pool.tile([1, 8], mm.float32, tag="osb")
                nc.vector.memset(osb[:], 0.0)
                nc.vector.tensor_copy(osb[:, 0:4], psF[:])
                nc.vector.tensor_copy(osb[:, 4:5], u2sb[:])
                nc.sync.dma_start(out=out_ext[:], in_=osb[:])

            if loop_n is None:
                emit_body()
            else:
                with tc.For_i(0, loop_n, 1):
                    emit_body()

    nc.compile()
    return nc


def shard_inputs(inputs: np.ndarray, targets: np.ndarray):
    in_maps = []
    for c in range(N_CORES):
        r, q = c // 4, c % 4
        mb = 2 * q + r
        ob = 2 * q + (1 - r)
        xq = np.concatenate(
            [inputs[:, 256 * mb:256 * (mb + 1)],
             inputs[:, 256 * ob:256 * (ob + 1)]], axis=1)
        tblocks = [mb] + [b for b in range(8) if b % 2 == r and b != mb]
        th = np.concatenate(
            [targets[:, 256 * b:256 * (b + 1)] for b in tblocks], axis=1)
        in_maps.append({
            "xq": np.ascontiguousarray(xq, dtype=np.float32),
            "th": np.ascontiguousarray(th, dtype=np.float32),
        })
    return in_maps


def combine_partials(outs) -> np.ndarray:
    """Host-side unshard: combine per-core [1,8] partials into the scalar."""
    D = float(B) * (B - 1)
    f = sum(float(o[0, 0]) for o in outs)
    p2 = sum(float(o[0, 1]) for o in outs)
    d = sum(float(o[0, 2]) for o in outs)
    m2 = sum(float(o[0, 3]) for o in outs)
    u2 = sum(float(o[0, 4]) for o in outs)
    loss = (ALPHA * f / (B * L)
            + (0.5 * u2 - 0.5 * p2 - m2 + d) / D)
    return np.float32(loss)


def kernel(inputs: np.ndarray, targets: np.ndarray) -> np.ndarray:
    if "nc" not in _CACHE:
        _CACHE["nc"] = build_nc()
    nc = _CACHE["nc"]
    in_maps = shard_inputs(np.asarray(inputs), np.asarray(targets))
    res = run_bass_kernel_spmd(nc, in_maps, list(range(N_CORES)))
    return combine_partials([res.results[c]["out"] for c in range(N_CORES)])


if __name__ == "__main__":
    rng = np.random.default_rng(0)
    x = rng.standard_normal((B, L)).astype(np.float32)
    t = (rng.random((B, L)) < 0.25).astype(np.float32)
    got = kernel(x, t)
    print("kernel out:", got)
